# revision 41
# baseline (speedup 1.0000x reference)
"""MoE feed-forward (top-2 of 8 experts, capacity-limited) for Trainium2.

Strategy (expert-parallel, matches the sharding hint):
  - Host (numpy): router softmax/top-k, capacity ranking, dispatch-table
    construction, token gather, and final scatter-add combine.  This is
    0.02% of the FLOPs.
  - Device (8 NeuronCores, SPMD): core e runs expert e's FFN over its
    gathered tokens:
        h = gelu(x_e @ W1[e] + b1[e]);  out = (h @ W2[e] + b2[e]) * g
    All activations live transposed ([feature, token]) so both matmuls
    use natural weight layouts and no on-device transposes are needed.
    Matmul inputs are bf16 (fp32 PSUM accumulation), ~3e-3 scale-relative
    output error vs the fp32 reference.
  - Occupied capacity slots are compacted at the front of each expert's
    block, so the device token dimension is sized per call to the max
    occupied count across experts (<= capacity 2560), skipping dead-slot
    compute; the Bass program is built/cached per token-tile plan.
"""

import math
from contextlib import ExitStack

import ml_dtypes
import numpy as np

import concourse.bass as bass
import concourse.mybir as mybir
import concourse.tile as tile
from concourse.bass import ds
from concourse.bass_utils import run_bass_kernel_spmd

# ---- problem constants (hardcoded per contract) ----
E, TOPK, CAP_FACTOR = 8, 2, 1.25
B, T, C = 4, 2048, 1024
H = 4 * C
N = B * T
NK = N * TOPK
CAPACITY = math.ceil(CAP_FACTOR * NK / E)  # 2560
NCORES = 8

TOK = CAPACITY  # tokens per expert (one core per expert)
KC = C // 128  # 8   k-subtiles for matmul 1
MH = H // 128  # 32  m-tiles for matmul 1 / k-subtiles for matmul 2
MC = C // 128  # 8   m-tiles for matmul 2

BF16 = mybir.dt.bfloat16
F32 = mybir.dt.float32
BF = ml_dtypes.bfloat16


def _split_multi_waits(nc, max_keep=1):
    """The walrus build in this container rejects instructions carrying more
    than ~2 sync waits; hoist extras onto preceding NoOps (same engine, same
    program position — semantically identical)."""
    for f in nc.m.functions:
        for blk in f.blocks:
            insts = list(blk.instructions)
            if not any(
                i.sync_info is not None
                and i.sync_info.on_wait
                and len(i.sync_info.on_wait) > max_keep
                for i in insts
            ):
                continue
            out = []
            for inst in insts:
                si = inst.sync_info
                ow = list(si.on_wait) if si is not None and si.on_wait else []
                if len(ow) > max_keep:
                    extra, keep = ow[:-max_keep], ow[-max_keep:]
                    for j, w in enumerate(extra):
                        out.append(
                            mybir.InstNoOp(
                                name=f"{inst.name}-waitsplit-{j}",
                                engine=inst.engine,
                                ins=[],
                                outs=[],
                                sync_info=mybir.SyncInfo(on_wait=[w], on_update=[]),
                            )
                        )
                    si.on_wait = keep
                out.append(inst)
            blk.instructions.clear()
            for i in out:
                blk.add_instruction(i)


def _tile_plan(maxcount):
    """Token-tile sizes covering the occupied slots, rounded up to 8.

    Remainders below 256 are rebalanced with the previous 512 tile into two
    roughly equal tiles: narrow matmuls pay proportionally more weight-load
    overhead on hardware."""
    tokd = min(max(32, -(-maxcount // 8) * 8), TOK)
    a, r = divmod(tokd, 512)
    if r == 0:
        sizes = [512] * a
    elif r >= 256 or a == 0:
        sizes = [512] * a + [r]
    else:
        s1 = -(-(512 + r) // 16) * 8  # half of (512+r), rounded up to 8
        sizes = [512] * (a - 1) + [s1, 512 + r - s1]
    return tuple(sizes)


W1CHUNK = 128  # H-columns per w1 chunk -> one m-tile, 0.25MB DMA
NW1 = H // W1CHUNK  # 32


def _build_bass(plan):
    TOKD = sum(plan)
    nc = bass.Bass("TRN2", target_bir_lowering=False, debug=False, num_devices=NCORES)
    # Inputs are pre-packed on the host into the exact SBUF layouts, so every
    # DMA below is a linear copy with multi-KB contiguous runs per partition.
    xt = nc.dram_tensor("xt", [128, KC * TOKD], BF16, kind="ExternalInput").ap()
    w1 = nc.dram_tensor("w1", [NW1, 128, KC * W1CHUNK], BF16, kind="ExternalInput").ap()
    w2 = nc.dram_tensor("w2", [MC, 128, MH * 128], BF16, kind="ExternalInput").ap()
    b1v = nc.dram_tensor("b1v", [128, MH], F32, kind="ExternalInput").ap()
    b2v = nc.dram_tensor("b2v", [128, MC], F32, kind="ExternalInput").ap()
    gr = nc.dram_tensor("gr", [128, TOKD], F32, kind="ExternalInput").ap()
    outT = nc.dram_tensor("outT", [C, TOKD], F32, kind="ExternalOutput").ap()

    out_r = outT.rearrange("(mo p) t -> p mo t", p=128)  # [128, MC, TOKD]

    gelu = mybir.ActivationFunctionType.Gelu
    ident = mybir.ActivationFunctionType.Identity

    with ExitStack() as ctx:
        tc = ctx.enter_context(tile.TileContext(nc))
        wpool = ctx.enter_context(tc.tile_pool(name="weights", bufs=1))
        xpool = ctx.enter_context(tc.tile_pool(name="x", bufs=2))
        hpool = ctx.enter_context(tc.tile_pool(name="h", bufs=1))
        opool = ctx.enter_context(tc.tile_pool(name="o", bufs=3))
        ps1 = ctx.enter_context(tc.tile_pool(name="ps1", bufs=3, space="PSUM"))
        ps2 = ctx.enter_context(tc.tile_pool(name="ps2", bufs=3, space="PSUM"))

        # PE warm-up: dependency-free matmuls on zeroed tiles keep the PE busy
        # (and the HAM clock un-throttled) while the first inputs DMA in, so
        # real matmuls start at full clock instead of paying the ramp.
        psd = ctx.enter_context(tc.tile_pool(name="psd", bufs=1, space="PSUM"))
        dum_w = wpool.tile([128, 128], BF16, tag="dum_w", name="dum_w")
        dum_x = wpool.tile([128, 512], BF16, tag="dum_x", name="dum_x")
        nc.gpsimd.memset(dum_w[:], 0.0)
        nc.gpsimd.memset(dum_x[:], 0.0)
        dum_ps = psd.tile([128, 512], F32, name="dum_ps")
        for _ in range(9):
            nc.tensor.matmul(dum_ps[:], dum_w[:], dum_x[:], start=True, stop=True)

        # Weights are loaded as per-column-chunk tiles so each matmul depends
        # only on its own chunk's DMA (per-tile dep tracking), and the DMA
        # stream is ordered so the first matmul's inputs land first.
        w1_tiles = []
        w2_tiles = []

        offsets = [sum(plan[:i]) for i in range(len(plan))]

        # Serial DMA stream order = consumption order: w1 chunk 0 and x tile 0
        # (first matmul), b1 (first gelu), remaining w1 chunks, then x tile 1
        # and the gates (needed ~60us+), then the w2 chunks.
        w1_tiles.append(wpool.tile([128, KC * W1CHUNK], BF16, tag="w1_0", name="w1_0"))
        nc.sync.dma_start(w1_tiles[0][:], w1[0])
        # x tile 0 is split into two k-halves so the first matmuls (k<4) only
        # wait on half the transfer
        KH = KC // 2
        x0h = []
        for h in range(2):
            x0t = xpool.tile([128, KH * plan[0]], BF16, tag=f"x0_{h}", name=f"x0_{h}")
            nc.sync.dma_start(x0t[:], xt[:, ds(h * KH * plan[0], KH * plan[0])])
            x0h.append(x0t)
        x_first = [None]
        b1_sb = wpool.tile([128, MH], F32)
        nc.sync.dma_start(b1_sb[:], b1v[:])
        b2_sb = wpool.tile([128, MC], F32)
        nc.sync.dma_start(b2_sb[:], b2v[:])
        for j in range(1, NW1):
            w1_tiles.append(wpool.tile([128, KC * W1CHUNK], BF16, tag=f"w1_{j}", name=f"w1_{j}"))
            nc.sync.dma_start(w1_tiles[j][:], w1[j])
        if len(plan) > 1:
            x_first.append(xpool.tile([128, KC * plan[1]], BF16, tag="x", name="x_1"))
            nc.sync.dma_start(x_first[1][:], xt[:, ds(KC * offsets[1], KC * plan[1])])
        g_sb = wpool.tile([128, TOKD], F32)
        nc.sync.dma_start(g_sb[:], gr[:])
        for j in range(MC):
            w2_tiles.append(wpool.tile([128, MH * 128], BF16, tag=f"w2_{j}", name=f"w2_{j}"))
            nc.sync.dma_start(w2_tiles[j][:], w2[j])

        MT_PER_CHUNK = W1CHUNK // 128  # m-tiles per w1 chunk

        for t, tsz in enumerate(plan):
            off = offsets[t]
            if t == 0:
                x_ap = lambda k: x0h[k // KH][:, ds((k % KH) * tsz, tsz)]
            else:
                if t < len(x_first):
                    x_sb = x_first[t]
                else:
                    x_sb = xpool.tile([128, KC * tsz], BF16, tag="x", name=f"x_{t}")
                    nc.sync.dma_start(x_sb[:], xt[:, ds(KC * off, KC * tsz)])
                x_ap = lambda k, x_sb=x_sb: x_sb[:, ds(k * tsz, tsz)]

            # hT[m-block, tokens] = gelu(W1[:, m-block].T @ xT + b1)
            h_sb = hpool.tile([128, MH, tsz], BF16, tag="h", name=f"h_{t}")
            for m in range(MH):
                w1c = w1_tiles[m // MT_PER_CHUNK]
                mo = m % MT_PER_CHUNK
                ps = ps1.tile([128, tsz], F32, tag="ps1", name=f"ps1_{t}_{m}")
                for k in range(KC):
                    nc.tensor.matmul(
                        ps[:],
                        w1c[:, ds(k * W1CHUNK + mo * 128, 128)],
                        x_ap(k),
                        start=(k == 0),
                        stop=(k == KC - 1),
                    )
                nc.scalar.activation(h_sb[:, m, :], ps[:], gelu, bias=b1_sb[:, m : m + 1])

            # outT[c-block, tokens] = (W2[:, c-block].T @ hT + b2) * g
            for m2 in range(MC):
                ps_o = ps2.tile([128, tsz], F32, tag="ps2", name=f"ps2_{t}_{m2}")
                for k2 in range(MH):
                    nc.tensor.matmul(
                        ps_o[:],
                        w2_tiles[m2][:, ds(k2 * 128, 128)],
                        h_sb[:, k2, :],
                        start=(k2 == 0),
                        stop=(k2 == MH - 1),
                    )
                ob = opool.tile([128, tsz], F32, tag="ob", name=f"ob_{t}_{m2}")
                nc.scalar.activation(ob[:], ps_o[:], ident, bias=b2_sb[:, m2 : m2 + 1])
                nc.vector.tensor_mul(ob[:], ob[:], g_sb[:, ds(off, tsz)])
                nc.sync.dma_start(out_r[:, m2, ds(off, tsz)], ob[:])

    _split_multi_waits(nc)
    return nc


_BASS_CACHE = {}
LAST_PLAN = (512, 512, 512, 512, 512)


def _get_bass(plan=(512, 512, 512, 512, 512)):
    if plan not in _BASS_CACHE:
        _BASS_CACHE[plan] = _build_bass(plan)
    return _BASS_CACHE[plan]


def _route(x_flat, Wr):
    """Replicates the reference router bit-for-bit in numpy fp32."""
    logits = x_flat @ Wr  # (N, E)
    mx = logits.max(axis=-1, keepdims=True)
    ex = np.exp(logits - mx)
    probs = ex / ex.sum(axis=-1, keepdims=True)
    order = np.argsort(-probs, axis=-1, kind="stable")  # jax top_k tie-break
    topk_idx = order[:, :TOPK]
    topk_vals = np.take_along_axis(probs, topk_idx, axis=-1)
    gates = topk_vals / topk_vals.sum(axis=-1, keepdims=True)

    idx_flat = topk_idx.reshape(-1)
    gates_flat = gates.reshape(-1)
    oh = (idx_flat[:, None] == np.arange(E)[None, :]).astype(np.int64)
    rank = np.take_along_axis(np.cumsum(oh, axis=0), idx_flat[:, None], axis=1)[:, 0]
    keep = rank <= CAPACITY
    ar = np.arange(NK)
    pos = np.where(
        (idx_flat[None, :] == np.arange(E)[:, None]) & keep[None, :], ar[None, :], NK
    )
    pos = np.sort(pos, axis=1)[:, :CAPACITY]  # (E, cap)
    valid = pos < NK
    pos_c = np.minimum(pos, NK - 1)
    n_idx = pos_c // TOPK
    kslot = pos_c % TOPK
    g = np.where(valid, gates_flat[pos_c], 0.0).astype(np.float32)

    assign_cnt = np.bincount(idx_flat, minlength=E)
    aux = np.float32(E * np.sum((assign_cnt / NK) * probs.mean(axis=0, dtype=np.float64)))
    return n_idx, kslot, valid, g, aux


def kernel(x, Wr, W1, b1, W2, b2):
    x = np.ascontiguousarray(np.asarray(x, dtype=np.float32))
    Wr = np.asarray(Wr, dtype=np.float32)
    W1 = np.asarray(W1, dtype=np.float32)
    b1 = np.asarray(b1, dtype=np.float32)
    W2 = np.asarray(W2, dtype=np.float32)
    b2 = np.asarray(b2, dtype=np.float32)

    x_flat = x.reshape(N, C)
    n_idx, kslot, valid, g, aux = _route(x_flat, Wr)

    # Occupied slots are compacted at the front of each expert's capacity
    # block; size the device token dim to the max occupied count (128-padded).
    maxcount = int(valid.sum(axis=1).max())
    plan = _tile_plan(maxcount)
    global LAST_PLAN
    LAST_PLAN = plan
    tokd = sum(plan)
    n_idx = n_idx[:, :tokd]
    kslot = kslot[:, :tokd]
    valid = valid[:, :tokd]
    g = g[:, :tokd]

    # dispatch: gather + pack into device SBUF layouts + cast per expert
    x_e = x_flat[n_idx]  # (E, tokd, C) fp32
    # xt[e]: [128, KC*tokd], tile-grouped: block t is [128, KC, tsz]
    xt_blocks = []
    off = 0
    for tsz in plan:
        blk = x_e[:, off : off + tsz, :].reshape(E, tsz, KC, 128)
        xt_blocks.append(blk.transpose(0, 3, 2, 1).reshape(E, 128, KC * tsz))
        off += tsz
    xt = np.ascontiguousarray(np.concatenate(xt_blocks, axis=2)).astype(BF)
    # w1[e]: [NW1, 128, KC*W1CHUNK]; w1b[j][p, k*W1CHUNK + c] = W1[k*128+p, j*W1CHUNK+c]
    w1b = np.ascontiguousarray(
        W1.reshape(E, KC, 128, NW1, W1CHUNK).transpose(0, 3, 2, 1, 4)
    ).reshape(E, NW1, 128, KC * W1CHUNK).astype(BF)
    # w2[e]: [MC, 128, MH*128]; w2b[j][p, k2*128 + c] = W2[k2*128+p, j*128+c]
    w2b = np.ascontiguousarray(
        W2.reshape(E, MH, 128, MC, 128).transpose(0, 3, 2, 1, 4)
    ).reshape(E, MC, 128, MH * 128).astype(BF)
    b1v = np.ascontiguousarray(b1.reshape(E, MH, 128).transpose(0, 2, 1)).astype(
        np.float32
    )
    b2v = np.ascontiguousarray(b2.reshape(E, MC, 128).transpose(0, 2, 1)).astype(
        np.float32
    )
    grs = np.ascontiguousarray(
        np.broadcast_to(g[:, None, :], (E, 128, tokd))
    ).astype(np.float32)

    nc = _get_bass(plan)
    in_maps = [
        {
            "xt": np.ascontiguousarray(xt[e]),
            "w1": np.ascontiguousarray(w1b[e]),
            "w2": np.ascontiguousarray(w2b[e]),
            "b1v": b1v[e],
            "b2v": b2v[e],
            "gr": grs[e],
        }
        for e in range(E)
    ]
    try:
        res = run_bass_kernel_spmd(nc, in_maps, core_ids=list(range(NCORES)))
    except Exception:
        # transient device/tunnel failure: one retry
        res = run_bass_kernel_spmd(nc, in_maps, core_ids=list(range(NCORES)))
    outT = np.stack([res.results[e]["outT"] for e in range(E)])  # (E, C, tokd)

    # combine: scatter-add gated expert outputs back to tokens
    out_rows = outT.transpose(0, 2, 1).reshape(-1, C)  # (E*cap, C)
    n_all = n_idx.reshape(-1)
    k_all = kslot.reshape(-1)
    v_all = valid.reshape(-1)
    y_flat = np.zeros((N, C), np.float32)
    for kk in range(TOPK):
        sel = v_all & (k_all == kk)
        buf = np.zeros((N, C), np.float32)
        buf[n_all[sel]] = out_rows[sel]
        y_flat += buf
    y = y_flat.reshape(B, T, C)
    return y, aux
